# revision 45
# baseline (speedup 1.0000x reference)
"""Multi-head attention (B=1, L=2048, D=1024, H=16) on 8 TRN2 NeuronCores.

Sharding: tensor-parallel over heads. Core i computes heads 2i, 2i+1:
  - projections with column shards of w_q/w_k/w_v (128 cols each)
  - full attention for its 2 heads
  - partial output projection with the matching 128-row shard of w_o
Host sums the 8 partial outputs and adds b_o once.

v2 design (vs 242us baseline at 235us): keep the PE HAM-warm and the
ScalarE (exp) stream continuous:
  - 16 junk matmuls at t=0 warm the PE clock (K=8/8) before real work
  - DMA order: weights, k (4 chunks), v-n0, q-half0 (4 chunks), v-n1..3,
    q-half1, wo; projection matmuls accumulate t-tile-paced so the PE
    never idles >3.4us during the load (HAM stays warm)
  - attention per (qh half, kt): scores ST[k,q] packed 2 heads in PE row
    groups; one exp per head [128,1024] on ScalarE (the critical engine:
    64 x 1.15us); AV with a ones-column appended to V (lhsT [128,65]) so
    row 64 of the PSUM accumulator collects the softmax denominators --
    no VectorE accumulation pass at all
  - v projection n-chunks 1..3 and q-half1 projection are emitted inside
    the qh0 attention loop (PE-queue order matches DMA arrival order)
  - vh layout via SBUF->SBUF DMA transposes (no PE transposes)
  - finalize: denominator rows spread-DMA'd to [128,2,16], one DVE
    reciprocal, DMA back; 1/d broadcast over 64 partitions with K=1
    matmuls; normalize on DVE; out-proj m-tiles with PSUM->SBUF copies
    alternating ScalarE/VectorE; output DMA alternating sync/gpsimd
"""

import os
import numpy as np
import ml_dtypes

import concourse.bass as bass
import concourse.mybir as mybir
import concourse.tile as tile
from concourse import bacc
from concourse.bass import ts
from concourse.bass_utils import run_bass_kernel_spmd
from concourse.masks import make_identity

P = 128
L = 2048
D = 1024
DH = 64
KT = D // P  # 8 contraction tiles
LT = L // P  # 16 seq tiles
NCORES = 8
BF16 = mybir.dt.bfloat16
F32 = mybir.dt.float32
AF = mybir.ActivationFunctionType
ALU = mybir.AluOpType

TRACE = False  # test.py flips this to get an NTFF profile / exec_time_ns
DEBUG = False  # dump intermediates to DRAM for host-side checking
LAST_RESULT = {}

_CACHED_NC = None


def _build():
    nc = bacc.Bacc("TRN2", target_bir_lowering=False, debug=False, num_devices=NCORES)

    qT = nc.dram_tensor("qT", [P, KT, L], BF16, kind="ExternalInput")
    kT = nc.dram_tensor("kT", [P, KT, L], BF16, kind="ExternalInput")
    vT = nc.dram_tensor("vT", [P, KT, L], BF16, kind="ExternalInput")
    wq = nc.dram_tensor("wq", [P, KT, P], BF16, kind="ExternalInput")
    wk = nc.dram_tensor("wk", [P, KT, P], BF16, kind="ExternalInput")
    wv = nc.dram_tensor("wv", [P, KT, P], BF16, kind="ExternalInput")
    bq = nc.dram_tensor("bq", [P, 1], F32, kind="ExternalInput")
    bk = nc.dram_tensor("bk", [P, 1], F32, kind="ExternalInput")
    bv = nc.dram_tensor("bv", [P, 1], F32, kind="ExternalInput")
    wo = nc.dram_tensor("wo", [P, D], BF16, kind="ExternalInput")
    out = nc.dram_tensor("out", [L, D], BF16, kind="ExternalOutput")
    dbg = {}
    if DEBUG:
        dbg["khT"] = nc.dram_tensor("d_khT", [P, L], BF16, kind="ExternalOutput")
        dbg["qhT"] = nc.dram_tensor("d_qhT", [P, L], BF16, kind="ExternalOutput")
        dbg["vhT"] = nc.dram_tensor("d_vhT", [P, L], BF16, kind="ExternalOutput")
        dbg["vh"] = nc.dram_tensor("d_vh", [P, LT * 2 * (DH + 1)], BF16, kind="ExternalOutput")
        dbg["u"] = nc.dram_tensor("d_u", [P, 2 * 1024], F32, kind="ExternalOutput")
        dbg["dall"] = nc.dram_tensor("d_dall", [1, 2 * 4 * 512], F32, kind="ExternalOutput")
        dbg["dallr"] = nc.dram_tensor("d_dallr", [1, 2 * 2 * 1024], BF16, kind="ExternalOutput")
        dbg["lhsT"] = nc.dram_tensor("d_lhsT", [P, L], BF16, kind="ExternalOutput")

    with tile.TileContext(nc) as tc:
        with (
            tc.tile_pool(name="const", bufs=1) as const_pool,
            tc.tile_pool(name="inputs", bufs=1) as in_pool,
            tc.tile_pool(name="proj", bufs=1) as sbp,
            tc.tile_pool(name="ptp", bufs=4) as pt_pool,
            tc.tile_pool(name="osbp", bufs=3) as osb_pool,
        ):
            ones_c = const_pool.tile([1, DH], BF16)
            nc.vector.memset(ones_c[:], 1.0)
            junk_w = const_pool.tile([P, 512], BF16)
            nc.vector.memset(junk_w[:], 0.0)
            identity = const_pool.tile([P, P], BF16)
            make_identity(nc, identity[:])
            # preload the exp activation-table set early (one-time ~2.7us)
            scr = const_pool.tile([1, 32], F32)
            nc.scalar.activation(scr[:], ones_c[0:1, 0:32], AF.Exp)

            wq_sb = in_pool.tile([P, KT, P], BF16)
            wk_sb = in_pool.tile([P, KT, P], BF16)
            wv_sb = in_pool.tile([P, KT, P], BF16)
            bq_sb = in_pool.tile([P, 1], F32)
            bk_sb = in_pool.tile([P, 1], F32)
            bv_sb = in_pool.tile([P, 1], F32)
            wo_sb = in_pool.tile([P, D], BF16)
            qT_sb = in_pool.tile([P, KT, L], BF16)
            kT_sb = in_pool.tile([P, KT, L], BF16)
            vT_sb = in_pool.tile([P, KT, L], BF16)

            qhT = sbp.tile([P, L], BF16)
            khT = sbp.tile([P, L], BF16)
            vhT = sbp.tile([P, L], BF16)
            vh = sbp.tile([P, LT, 2, DH + 1], BF16)  # [kseq, kt, head, dh|ones]
            nc.vector.memset(vh[:, :, :, DH : DH + 1], 1.0)
            u_all = sbp.tile([P, 2, 1024], F32)  # unnormalized concat^T per qh
            lhsT_c = sbp.tile([P, L], BF16)  # normalized concat^T
            dall_sb = sbp.tile([1, 2, 4, 512], F32)  # d rows [qh, (h,j), q]
            dsp = sbp.tile([P, 2, 16], F32)  # spread denominators per qh
            dspb = sbp.tile([P, 2, 16], BF16)
            dallr = sbp.tile([1, 2, 2, 1024], BF16)  # 1/d [qh, h, q]

            # ---- DMA issue: stripe across all 3 DMA-capable queues
            # (sync/scalar/gpsimd). ScalarE only issues (~1.4us each) before
            # the exp stream starts, so its queue is free to help the load.
            # t-granular chunks so projections accumulate as data arrives. ----
            nc.sync.dma_start(wq_sb[:], wq[:])
            nc.sync.dma_start(bq_sb[:], bq[:])
            nc.scalar.dma_start(wk_sb[:], wk[:])
            nc.scalar.dma_start(bk_sb[:], bk[:])
            nc.gpsimd.dma_start(wv_sb[:], wv[:])
            nc.gpsimd.dma_start(bv_sb[:], bv[:])
            qrr = (nc.sync, nc.scalar, nc.gpsimd)

            def v_chunk(n):  # one n-chunk of v, striped over the 3 queues by t
                for i, (a, b) in enumerate(((0, 3), (3, 6), (6, 8))):
                    qrr[i].dma_start(
                        vT_sb[:, a:b, ts(n, 512)], vT[:, a:b, ts(n, 512)]
                    )

            # half-t chunks (0.25MB): fine-grained arrival keeps the
            # t-paced projection matmuls dense enough that HAM stays warm
            for c in range(2 * KT):
                t, half = c // 2, c % 2
                qrr[c % 3].dma_start(
                    kT_sb[:, t, ts(half, 1024)], kT[:, t, ts(half, 1024)]
                )
            v_chunk(0)
            for c in range(2 * KT):
                t, half = c // 2, c % 2
                qrr[c % 3].dma_start(
                    qT_sb[:, t, half * 512 : (half + 1) * 512],
                    qT[:, t, half * 512 : (half + 1) * 512],
                )
            for n in (1, 2, 3):
                v_chunk(n)
            nc.sync.dma_start(qT_sb[:, 0:4, 1024:2048], qT[:, 0:4, 1024:2048])
            nc.gpsimd.dma_start(qT_sb[:, 4:8, 1024:2048], qT[:, 4:8, 1024:2048])
            nc.gpsimd.dma_start(wo_sb[:], wo[:])

            # ---- PE warmup: get HAM to K=8/8 and bridge to the first
            # DMA-paced projection matmuls ----
            with tc.tile_pool(name="psJ", bufs=1, space="PSUM") as psJ:
                jp = psJ.tile([P, 512], F32)
                for i in range(36):
                    nc.tensor.matmul(
                        jp[:], junk_w[:, 0:P], junk_w[:], start=True, stop=True
                    )

            def vproj(vp, trs, n):
                """v projection for n-chunk (512 seq cols) + vh transposes.
                Tiles are pre-allocated by the caller so the av-tag buffer
                cycle stays dependency-forward."""
                for t in range(KT):
                    nc.tensor.matmul(
                        vp[:],
                        wv_sb[:, t, :],
                        vT_sb[:, t, ts(n, 512)],
                        start=(t == 0),
                        stop=(t == KT - 1),
                    )
                nc.vector.tensor_scalar(
                    vhT[:, ts(n, 512)], vp[:], bv_sb[:], None, op0=ALU.add
                )
                for i, t2 in enumerate(range(4 * n, 4 * n + 4)):
                    # PE-mode transpose (the xbar DMA transpose costs 1.24us
                    # of HWDGE queue time each), then a strided copy into the
                    # [head, dh|ones] layout
                    nc.tensor.transpose(trs[i][:], vhT[:, ts(t2, P)], identity[:])
                    nc.vector.tensor_copy(
                        vh[:, t2, :, 0:DH],
                        trs[i][:].rearrange("p (h d) -> p h d", h=2),
                    )

            with tc.tile_pool(name="ps", bufs=2, space="PSUM") as ps:
                # ---- k projection: t-tile-paced into 2 [P,1024] psum tiles
                k01 = ps.tile([P, 1024], F32, tag="st", name="k01")
                k23 = ps.tile([P, 1024], F32, tag="st", name="k23")
                for t in range(KT):
                    for half, pst in ((0, k01), (1, k23)):
                        for j in (0, 1):
                            nc.tensor.matmul(
                                pst[:, ts(j, 512)],
                                wk_sb[:, t, :],
                                kT_sb[:, t, half * 1024 + j * 512 : half * 1024 + (j + 1) * 512],
                                start=(t == 0),
                                stop=(t == KT - 1),
                            )
                nc.vector.tensor_scalar(
                    khT[:, 0:1024], k01[:], bk_sb[:], None, op0=ALU.add
                )
                nc.vector.tensor_scalar(
                    khT[:, 1024:2048], k23[:], bk_sb[:], None, op0=ALU.add
                )

                def av_alloc(nm):
                    return ps.tile([P, 512], F32, tag="av", bufs=4, name=nm)

                def tr_alloc(n, tag):
                    b = 4 if tag == "av" else 2
                    return [
                        ps.tile([P, P], BF16, tag=tag, bufs=b, name=f"tr{n}_{i}")
                        for i in range(4)
                    ]

                def st_alloc(nm):
                    return ps.tile([P, 1024], F32, tag="st", name=nm)

                # ---- v projection, n-chunk 0 (av-tag psum; its DMA lands
                # before q half0 finishes) ----
                vproj(av_alloc("vp0"), tr_alloc(0, "av"), 0)

                # ---- q projection, half 0 ----
                q0 = st_alloc("q0")
                for t in range(KT):
                    for j in (0, 1):
                        nc.tensor.matmul(
                            q0[:, ts(j, 512)],
                            wq_sb[:, t, :],
                            qT_sb[:, t, ts(j, 512)],
                            start=(t == 0),
                            stop=(t == KT - 1),
                        )
                nc.vector.tensor_scalar(
                    qhT[:, 0:1024], q0[:], bq_sb[:], None, op0=ALU.add
                )

                # ---- attention: 2 q-halves x 16 kseq tiles. v-proj chunks
                # 1..3 are emitted inside the qh0 loop (st-tag psum) as their
                # DMAs land, so attention starts without waiting for v. ----
                for qh in (0, 1):
                    avt = {}
                    for h in (0, 1):
                        for j in (0, 1):
                            avt[h, j] = av_alloc(f"av{qh}{h}{j}")
                    for kt in range(LT):
                        sts = {}
                        for h in (0, 1):
                            sts[h] = ps.tile(
                                [P, 1024], F32, tag="st", name=f"st{qh}_{kt}_{h}"
                            )
                        # j-major: adjacent matmuls use different PE row
                        # groups (h0 rows 0-63, h1 rows 64-127) so they can
                        # stream concurrently in the array
                        for j in (0, 1):
                            for h in (0, 1):
                                nc.tensor.matmul(
                                    sts[h][:, ts(j, 512)],
                                    khT[ts(h, DH), ts(kt, P)],
                                    qhT[ts(h, DH), qh * 1024 + j * 512 : qh * 1024 + (j + 1) * 512],
                                )
                        pts = {}
                        for h in (0, 1):
                            pt = pt_pool.tile(
                                [P, 1024], BF16, tag="pt", name=f"pt{qh}_{kt}_{h}"
                            )
                            nc.scalar.activation(pt[:], sts[h][:], AF.Exp, scale=0.125)
                            pts[h] = pt
                        if qh == 0 and kt in (0, 1, 2):
                            # v-proj chunk kt+1: st-tag psum, slotted into the
                            # PE stream between this kt's scores and AVs
                            vproj(
                                st_alloc(f"vp{kt + 1}")[:, 0:512],
                                tr_alloc(kt + 1, "st"),
                                kt + 1,
                            )
                        if qh == 0 and kt == 6:
                            # q projection, half 1 (DMA landed long before)
                            q1 = st_alloc("q1")
                            for t in range(KT):
                                for j in (0, 1):
                                    nc.tensor.matmul(
                                        q1[:, ts(j, 512)],
                                        wq_sb[:, t, :],
                                        qT_sb[:, t, 1024 + j * 512 : 1024 + (j + 1) * 512],
                                        start=(t == 0),
                                        stop=(t == KT - 1),
                                    )
                            nc.vector.tensor_scalar(
                                qhT[:, 1024:2048], q1[:], bq_sb[:], None, op0=ALU.add
                            )
                        for h in (0, 1):
                            for j in (0, 1):
                                nc.tensor.matmul(
                                    avt[h, j][0 : DH + 1, :],
                                    vh[:, kt, h, 0 : DH + 1],
                                    pts[h][:, ts(j, 512)],
                                    start=(kt == 0),
                                    stop=(kt == LT - 1),
                                )
                    # drain this q-half: U rows to SBUF, denominator row
                    # spread + reciprocal (qh0's chain overlaps qh1's loop;
                    # qh1's d-copies go on ScalarE, idle once exps are done)
                    for h in (0, 1):
                        for j in (0, 1):
                            nc.vector.tensor_copy(
                                u_all[ts(h, DH), qh, ts(j, 512)], avt[h, j][0:DH, :]
                            )
                            c = 2 * h + j
                            if qh == 0:
                                nc.vector.tensor_copy(
                                    dall_sb[0:1, qh, c, :], avt[h, j][DH : DH + 1, :]
                                )
                            else:
                                nc.scalar.copy(
                                    dall_sb[0:1, qh, c, :], avt[h, j][DH : DH + 1, :]
                                )
                            (nc.sync if c % 2 == 0 else nc.gpsimd).dma_start(
                                dsp[32 * c : 32 * c + 32, qh, :], dall_sb[0:1, qh, c, :]
                            )
                    nc.vector.reciprocal(dsp[:, qh, :], dsp[:, qh, :])
                    nc.vector.tensor_copy(dspb[:, qh, :], dsp[:, qh, :])
                    (nc.sync if qh == 0 else nc.gpsimd).dma_start(
                        dallr[0:1, qh, :, :], dspb[:, qh, :]
                    )

                # PE keep-warm filler for the qh1 drain lull (ready once its
                # st slot frees; scheduled only when attention MMs are done)
                jfill = st_alloc("jfill")
                for i in range(8):
                    nc.tensor.matmul(
                        jfill[:, 0:512], junk_w[:, 0:P], junk_w[:], start=True, stop=True
                    )

            if DEBUG:
                nc.gpsimd.dma_start(dbg["khT"][:], khT[:])
                nc.gpsimd.dma_start(dbg["qhT"][:], qhT[:])
                nc.gpsimd.dma_start(dbg["vhT"][:], vhT[:])
                nc.gpsimd.dma_start(
                    dbg["vh"][:], vh.rearrange("p a b c -> p (a b c)")
                )
                nc.gpsimd.dma_start(dbg["u"][:], u_all.rearrange("p a b -> p (a b)"))
                nc.gpsimd.dma_start(
                    dbg["dall"][:], dall_sb.rearrange("p a b c -> p (a b c)")
                )
                nc.gpsimd.dma_start(
                    dbg["dallr"][:], dallr.rearrange("p a b c -> p (a b c)")
                )

            # ---- finalize: broadcast 1/d, normalize, output projection ----
            with tc.tile_pool(name="psC", bufs=2, space="PSUM") as psC:
                # pre-allocate the out-proj psum tiles FIRST so the "op" tag
                # lands on the attention pool's st banks (which drain at the
                # last exp) instead of behind the qh1 avt drain chain
                ops = [
                    psC.tile([P, D], F32, tag="op", name=f"op{m}") for m in range(LT)
                ]
                # PE keep-warm bridge over the qh1 reciprocal round-trip:
                # K=1 matmuls gated on qh1's 1/d landing (true tail lull)
                jnk = psC.tile([P, 512], F32, tag="jnk", bufs=1)
                for i in range(8):
                    nc.tensor.matmul(
                        jnk[:],
                        junk_w[0:1, 0:P],
                        dallr[0:1, 1, 0, 0:512],
                        start=True,
                        stop=True,
                    )

                def finalize_qh(qh):
                    for j in (0, 1):
                        bc = psC.tile([P, 512], F32, tag="fin", name=f"bc{qh}{j}")
                        for h in (0, 1):
                            nc.tensor.matmul(
                                bc[ts(h, DH), :],
                                ones_c[0:1, :],
                                dallr[0:1, qh, h, ts(j, 512)],
                                tile_position=(0, DH * h),
                            )
                        nc.vector.tensor_tensor(
                            lhsT_c[:, qh * 1024 + j * 512 : qh * 1024 + (j + 1) * 512],
                            u_all[:, qh, ts(j, 512)],
                            bc[:],
                            op=ALU.mult,
                        )

                osb_live = {}

                def outproj(ms):
                    for m in ms:
                        op = ops[m]
                        for n in (0, 1):
                            nc.tensor.matmul(
                                op[:, ts(n, 512)],
                                lhsT_c[:, ts(m, P)],
                                wo_sb[:, ts(n, 512)],
                            )
                        if m % 2 == 0:
                            osb = osb_pool.tile(
                                [P, 2, D], BF16, tag="osb", bufs=2, name=f"osb{m // 2}"
                            )
                            osb_live[m + 1] = osb
                        else:
                            osb = osb_live[m]
                        # split the PSUM->SBUF cast across both free engines
                        nc.vector.tensor_copy(osb[:, m % 2, 0:512], op[:, 0:512])
                        nc.scalar.copy(osb[:, m % 2, 512:1024], op[:, 512:1024])
                        if m % 2 == 1:
                            # two row-tiles per DMA (fewer 1.4us issue slots)
                            dst = out[(m - 1) * P : (m + 1) * P, :].rearrange(
                                "(i p) d -> p i d", i=2
                            )
                            qrr[(m // 2) % 3].dma_start(dst, osb[:])

                finalize_qh(0)
                outproj(range(0, 8))
                finalize_qh(1)
                outproj(range(8, 16))
                if DEBUG:
                    nc.gpsimd.dma_start(dbg["lhsT"][:], lhsT_c[:])

    nc.compile()
    return nc


def kernel(q, k, v, w_q, b_q, w_k, b_k, w_v, b_v, w_o, b_o):
    global _CACHED_NC, LAST_RESULT
    if _CACHED_NC is None:
        _CACHED_NC = _build()
    nc = _CACHED_NC

    bf16 = ml_dtypes.bfloat16

    def tile_T(x):  # [1, L, D] -> [128, D//128, L] contiguous
        xt = np.asarray(x, np.float32)[0].T  # [D, L]
        return np.ascontiguousarray(
            xt.reshape(D // P, P, L).transpose(1, 0, 2)
        ).astype(bf16)

    def tile_w(w):  # [D, 128] -> [128, D//128, 128] contiguous
        return np.ascontiguousarray(
            w.reshape(D // P, P, P).transpose(1, 0, 2)
        ).astype(bf16)

    q2 = tile_T(q)
    k2 = tile_T(k)
    v2 = tile_T(v)
    w_q = np.asarray(w_q, np.float32)
    w_k = np.asarray(w_k, np.float32)
    w_v = np.asarray(w_v, np.float32)
    w_o = np.asarray(w_o, np.float32)
    b_q = np.asarray(b_q, np.float32)
    b_k = np.asarray(b_k, np.float32)
    b_v = np.asarray(b_v, np.float32)
    b_o = np.asarray(b_o, np.float32)

    in_maps = []
    for i in range(NCORES):
        sl = slice(P * i, P * (i + 1))
        in_maps.append(
            {
                "qT": q2,
                "kT": k2,
                "vT": v2,
                "wq": tile_w(w_q[:, sl]),
                "wk": tile_w(w_k[:, sl]),
                "wv": tile_w(w_v[:, sl]),
                "bq": np.ascontiguousarray(b_q[sl]).reshape(P, 1),
                "bk": np.ascontiguousarray(b_k[sl]).reshape(P, 1),
                "bv": np.ascontiguousarray(b_v[sl]).reshape(P, 1),
                "wo": np.ascontiguousarray(w_o[sl, :]).astype(bf16),
            }
        )

    kwargs = {}
    if TRACE:
        import shutil

        tdir = "/tmp/bass_trace"
        shutil.rmtree(tdir, ignore_errors=True)
        os.makedirs(tdir, exist_ok=True)
        kwargs["tmpdir"] = tdir
    res = run_bass_kernel_spmd(nc, in_maps, list(range(NCORES)), trace=TRACE, **kwargs)
    LAST_RESULT = {
        "exec_time_ns": res.exec_time_ns,
        "trace_path": (res.instructions_and_trace or (None, None))[1],
    }
    acc = np.zeros((L, D), np.float64)
    for i in range(NCORES):
        acc += res.results[i]["out"].astype(np.float64)
    acc += b_o.astype(np.float64)
    return acc.astype(np.float32).reshape(1, L, D)
